# revision 12
# baseline (speedup 1.0000x reference)
"""AriaText MoE layer on 8 Trainium2 NeuronCores.

Strategy (expert-parallel + token-sharded shared expert):
- Host: router (softmax/top-4/renorm), per-expert token gather (pre-transposed
  activations), weight retile + bf16 cast, one-hot dispatch/combine matrices.
  Experts are paired onto cores big-with-small to minimize the padded
  capacity C.
- Device, per core (2 experts; token shard of 256):
  * gate/up/down GEMMs for the core's experts over their routed tokens
    (padded to capacity C), bf16 compute with fp32 PSUM accumulation;
    combine weight applied via per-partition scale on the PSUM->SBUF copy.
  * per 512-column slice: one-hot matmul reorders weighted expert rows into
    an AllToAll send slab [dst core][slot]; 5 column-sliced bf16 AllToAlls
    pipeline behind the down-projection so collective DMA traffic never
    stalls the TensorEngine.
  * shared expert computed token-sharded (full FS intermediate, 256 tokens).
  * final PSUM chain per output tile: shared-expert down-proj + one-hot
    scatter-add of received expert rows -> [256, 2560] f32 shard.
- Host concatenates the 8 shards into the full [1, 2048, 2560] output.
"""

import numpy as np
import ml_dtypes

import concourse.mybir as mybir
import concourse.tile as tile
from concourse import bacc
from concourse.bass_utils import run_bass_kernel_spmd

E, TOPK, D, F, FS = 16, 4, 2560, 1664, 3328
T = 2048
NC = 8
TS = T // NC  # tokens per core
EPC = E // NC  # experts per core
FT = F // 128  # 13
DT = D // 128  # 20
FST = FS // 128  # 26
DC = D // 512  # 5 output column chunks
TT = TS // 128  # 2 token tiles per core
BF16 = mybir.dt.bfloat16
F32 = mybir.dt.float32
AF = mybir.ActivationFunctionType


def _route(x32, router_weight):
    """Replicate reference routing (f64 for a stable top-k ordering)."""
    lg = x32.astype(np.float64) @ router_weight.astype(np.float64).T
    lg -= lg.max(-1, keepdims=True)
    p = np.exp(lg)
    p /= p.sum(-1, keepdims=True)
    idx = np.argsort(-p, axis=-1, kind="stable")[:, :TOPK]
    w = np.take_along_axis(p, idx, axis=-1)
    w = w / w.sum(-1, keepdims=True)
    return idx, w.astype(np.float32)


def _build(Cs, NSLOT, send_pairs, recv_pairs):
    """Build the SPMD graph. Cs = per-local-expert token capacities,
    NSLOT = A2A slab rows. send_pairs[st] = [(el, ct), ...];
    recv_pairs[tt] = [ct, ...] (union over cores, identical graph)."""
    C = max(Cs)
    CTs = [(c + 127) // 128 for c in Cs]
    CT = sum(CTs)
    cto = [0, CTs[0]]  # wvt column offset per el
    NCT = NSLOT // 128
    nc = bacc.Bacc("TRN2", target_bir_lowering=False, debug=False, num_devices=NC)

    xg_in = nc.dram_tensor("xg", [EPC, D, C], BF16, kind="ExternalInput")
    xs_in = nc.dram_tensor("xs", [D, TS], BF16, kind="ExternalInput")
    wg_in = nc.dram_tensor("wg", [EPC, FT, 128, DT, 128], BF16, kind="ExternalInput")
    wu_in = nc.dram_tensor("wu", [EPC, FT, 128, DT, 128], BF16, kind="ExternalInput")
    wd_in = nc.dram_tensor("wd", [EPC, FT, 128, D], BF16, kind="ExternalInput")
    swg_in = nc.dram_tensor("swg", [FST, 128, DT, 128], BF16, kind="ExternalInput")
    swu_in = nc.dram_tensor("swu", [FST, 128, DT, 128], BF16, kind="ExternalInput")
    swd_in = nc.dram_tensor("swd", [FST, 128, D], BF16, kind="ExternalInput")
    wvt_in = nc.dram_tensor("wvt", [128, CT], F32, kind="ExternalInput")
    ohs_in = nc.dram_tensor("ohs", [EPC, max(CTs), 128, NSLOT], BF16, kind="ExternalInput")
    ohr_in = nc.dram_tensor("ohr", [NCT, 128, TS], BF16, kind="ExternalInput")
    out_ext = nc.dram_tensor("out", [TS, D], F32, kind="ExternalOutput")

    chunks_el = []
    for c in Cs:
        ch = [(0, min(512, c))]
        if c > 512:
            ch.append((512, c - 512))
        chunks_el.append(ch)
    cth_el = [
        [min(128, c - ct * 128) for ct in range(n)] for c, n in zip(Cs, CTs)
    ]  # per-el per-c-tile height

    n_ohp = sum(len(p) for p in send_pairs)

    with tile.TileContext(nc) as tc:
        with (
            tc.tile_pool(name="sb", bufs=1) as sb,
            tc.tile_pool(name="ps", bufs=1, space="PSUM") as ps,
            tc.tile_pool(name="dr", bufs=1, space="DRAM") as dr,
        ):
            send_slabs = [
                dr.tile([NSLOT, 512], BF16, tag="slab", bufs=DC, name=f"sslab{dc}")
                for dc in range(DC)
            ]
            recv_slabs = [
                dr.tile([NSLOT, 512], BF16, tag="rslab", bufs=DC, name=f"rslab{dc}")
                for dc in range(DC)
            ]

            wvt_sb = sb.tile([128, CT], F32, tag="wvt", bufs=1, name="wvt")
            nc.sync.dma_start(wvt_sb[:], wvt_in[:])

            # tiny collective up-front: absorbs the first-collective
            # cross-core rendezvous off the critical path
            warm_s = dr.tile([NC, 512], BF16, tag="warm", bufs=2, name="warms")
            warm_r = dr.tile([NC, 512], BF16, tag="warm", bufs=2, name="warmr")
            warm_sb = sb.tile([NC, 512], BF16, tag="ssb", bufs=3, name="warmsb")
            nc.vector.memset(warm_sb[:], 0.0)
            nc.sync.dma_start(warm_s[:], warm_sb[:])
            nc.gpsimd.collective_compute(
                "AllToAll",
                mybir.AluOpType.bypass,
                replica_groups=[list(range(NC))],
                ins=[warm_s.opt()],
                outs=[warm_r.opt()],
            )

            # ---- phase 1a: gate/up for both experts ----
            h_tiles = {}  # (el, fi) -> tile [128, C_el]
            xg_sb = {}
            for el in range(EPC):
                Ce = Cs[el]
                chunks = chunks_el[el]
                # first weight slab lands before the xg block: shortens the head
                wgu_pre = {}
                wgt0 = sb.tile([128, DT, 128], BF16, tag="wgu", bufs=9, name=f"wg{el}_0")
                nc.sync.dma_start(wgt0[:], wg_in[el, 0])
                wut0 = sb.tile([128, DT, 128], BF16, tag="wgu", bufs=9, name=f"wu{el}_0")
                nc.sync.dma_start(wut0[:], wu_in[el, 0])
                wgu_pre[0] = (wgt0, wut0)
                for dt in range(DT):
                    t_ = sb.tile([128, Ce], BF16, tag="xgt", bufs=21, name=f"xg{el}_{dt}")
                    nc.sync.dma_start(t_[:], xg_in[el, dt * 128 : (dt + 1) * 128, :Ce])
                    xg_sb[(el, dt)] = t_
                for fi in range(FT):
                    if fi in wgu_pre:
                        wgt, wut = wgu_pre[fi]
                    else:
                        wgt = sb.tile([128, DT, 128], BF16, tag="wgu", bufs=9, name=f"wg{el}_{fi}")
                        nc.sync.dma_start(wgt[:], wg_in[el, fi])
                        wut = sb.tile([128, DT, 128], BF16, tag="wgu", bufs=9, name=f"wu{el}_{fi}")
                        nc.sync.dma_start(wut[:], wu_in[el, fi])
                    h_t = sb.tile([128, Ce], BF16, tag="h", bufs=2 * FT + 2, name=f"h{el}_{fi}")
                    pgs = [
                        ps.tile([128, 512], F32, tag="pgu", bufs=6, name=f"pg{el}_{fi}_{i}")
                        for i in range(len(chunks))
                    ]
                    pus = [
                        ps.tile([128, 512], F32, tag="pgu", bufs=6, name=f"pu{el}_{fi}_{i}")
                        for i in range(len(chunks))
                    ]
                    # interleave: same stationary drives all chunks back-to-back
                    for dt in range(DT):
                        st_, sp_ = (dt == 0), (dt == DT - 1)
                        for i, (off, cw) in enumerate(chunks):
                            nc.tensor.matmul(
                                pgs[i][:, :cw], wgt[:, dt, :],
                                xg_sb[(el, dt)][:, off : off + cw],
                                start=st_, stop=sp_,
                            )
                        for i, (off, cw) in enumerate(chunks):
                            nc.tensor.matmul(
                                pus[i][:, :cw], wut[:, dt, :],
                                xg_sb[(el, dt)][:, off : off + cw],
                                start=st_, stop=sp_,
                            )
                    for i, (off, cw) in enumerate(chunks):
                        sg = sb.tile([128, 512], BF16, tag="sg", bufs=2, name=f"sg{el}_{fi}_{i}")
                        nc.scalar.activation(sg[:, :cw], pgs[i][:, :cw], AF.Silu)
                        nc.vector.tensor_mul(h_t[:, off : off + cw], sg[:, :cw], pus[i][:, :cw])
                    h_tiles[(el, fi)] = h_t

            # ---- phase 1b: per column-slice: down-proj both experts,
            #      one-hot reorder into the send slab, column-sliced AllToAll ----
            ohp_tiles = {}
            for st in range(NCT):
                for el, ct in send_pairs[st]:
                    oh_t = sb.tile(
                        [128, 128], BF16, tag="ohp", bufs=n_ohp + 1,
                        name=f"ohp{st}_{el}_{ct}",
                    )
                    nc.sync.dma_start(oh_t[:], ohs_in[el, ct, :, st * 128 : (st + 1) * 128])
                    ohp_tiles[(st, el, ct)] = oh_t

            for dc in range(DC):
                y_sb = {}
                for el in range(EPC):
                    cth = cth_el[el]
                    wd_sl = []
                    for fi in range(FT):
                        t_ = sb.tile([128, 512], BF16, tag="wd", bufs=38, name=f"wd{el}_{fi}_{dc}")
                        nc.sync.dma_start(t_[:], wd_in[el, fi, :, dc * 512 : (dc + 1) * 512])
                        wd_sl.append(t_)
                    # interleave pairs of ct-chains so weight loads overlap matmuls
                    cts = list(range(CTs[el]))
                    for j in range(0, CTs[el], 2):
                        grp = cts[j : j + 2]
                        pys = {
                            ct: ps.tile([128, 512], F32, tag="py", bufs=2, name=f"py{el}_{dc}_{ct}")
                            for ct in grp
                        }
                        for fi in range(FT):
                            for ct in grp:
                                nc.tensor.matmul(
                                    pys[ct][: cth[ct], :],
                                    h_tiles[(el, fi)][:, ct * 128 : ct * 128 + cth[ct]],
                                    wd_sl[fi][:],
                                    start=(fi == 0), stop=(fi == FT - 1),
                                )
                        for ct in grp:
                            ysb = sb.tile([128, 512], BF16, tag="y", bufs=11, name=f"y{el}_{dc}_{ct}")
                            nc.scalar.activation(
                                ysb[: cth[ct], :], pys[ct][: cth[ct], :], AF.Copy,
                                scale=wvt_sb[: cth[ct], cto[el] + ct : cto[el] + ct + 1],
                            )
                            y_sb[(el, ct)] = ysb
                for st in range(NCT):
                    pairs = send_pairs[st]
                    ps_ = ps.tile([128, 512], F32, tag="py", bufs=2, name=f"pss{st}_{dc}")
                    for i, (el, ct) in enumerate(pairs):
                        h_ = cth_el[el][ct]
                        nc.tensor.matmul(
                            ps_[:], ohp_tiles[(st, el, ct)][:h_, :], y_sb[(el, ct)][:h_, :],
                            start=(i == 0), stop=(i == len(pairs) - 1),
                        )
                    ssb = sb.tile([128, 512], BF16, tag="ssb", bufs=3, name=f"ss{st}_{dc}")
                    nc.scalar.activation(ssb[:], ps_[:], AF.Copy)
                    nc.sync.dma_start(send_slabs[dc][st * 128 : (st + 1) * 128, :], ssb[:])
                nc.gpsimd.collective_compute(
                    "AllToAll",
                    mybir.AluOpType.bypass,
                    replica_groups=[list(range(NC))],
                    ins=[send_slabs[dc].opt()],
                    outs=[recv_slabs[dc].opt()],
                )

            # ---- phase 2: shared expert gate/up (token-sharded) ----
            xs_sb = sb.tile([128, DT, TS], BF16, tag="xs", bufs=1, name="xs")
            nc.sync.dma_start(xs_sb[:], xs_in.rearrange("(n p) t -> p n t", p=128))
            hs_tiles = []
            for fi in range(FST):
                sgt = sb.tile([128, DT, 128], BF16, tag="wgu", bufs=9, name=f"swg{fi}")
                nc.sync.dma_start(sgt[:], swg_in[fi])
                sut = sb.tile([128, DT, 128], BF16, tag="wgu", bufs=9, name=f"swu{fi}")
                nc.sync.dma_start(sut[:], swu_in[fi])
                pg = ps.tile([128, 512], F32, tag="pgu", bufs=6, name=f"psg{fi}")
                pu = ps.tile([128, 512], F32, tag="pgu", bufs=6, name=f"psu{fi}")
                for dt in range(DT):
                    st_, sp_ = (dt == 0), (dt == DT - 1)
                    nc.tensor.matmul(
                        pg[:, :TS], sgt[:, dt, :], xs_sb[:, dt, :], start=st_, stop=sp_
                    )
                    nc.tensor.matmul(
                        pu[:, :TS], sut[:, dt, :], xs_sb[:, dt, :], start=st_, stop=sp_
                    )
                sg = sb.tile([128, 512], BF16, tag="sg", bufs=2, name=f"ssg{fi}")
                nc.scalar.activation(sg[:, :TS], pg[:, :TS], AF.Silu)
                hs_t = sb.tile([128, TS], BF16, tag="hs", bufs=FST + 1, name=f"hs{fi}")
                nc.vector.tensor_mul(hs_t[:], sg[:, :TS], pu[:, :TS])
                hs_tiles.append(hs_t)

            # ---- phase 3: shared down-proj + scatter of received expert rows ----
            ohr_sb = []
            for ct in range(NCT):
                t_ = sb.tile([128, TS], BF16, tag="ohr", bufs=NCT + 1, name=f"ohr{ct}")
                nc.sync.dma_start(t_[:], ohr_in[ct])
                ohr_sb.append(t_)
            for dc in range(DC):
                rcv_tiles = {}
                for tt in range(TT):
                    for ct in recv_pairs[tt]:
                        if ct not in rcv_tiles:
                            rt = sb.tile([128, 512], BF16, tag="rcv", bufs=9, name=f"rcv{ct}_{dc}")
                            nc.sync.dma_start(
                                rt[:], recv_slabs[dc][ct * 128 : (ct + 1) * 128, :]
                            )
                            rcv_tiles[ct] = rt
                pos = {
                    tt: ps.tile([128, 512], F32, tag="py", bufs=2, name=f"po{dc}_{tt}")
                    for tt in range(TT)
                }
                n_mm = {tt: FST + len(recv_pairs[tt]) for tt in range(TT)}
                idx = {tt: 0 for tt in range(TT)}
                # swd slices in two halves to bound SBUF (shared "wd" tag)
                for half in range(2):
                    fis = range(13 * half, min(13 * (half + 1), FST))
                    sl = {}
                    for fi in fis:
                        t_ = sb.tile([128, 512], BF16, tag="wd", bufs=38, name=f"swd{fi}_{dc}")
                        nc.sync.dma_start(t_[:], swd_in[fi, :, dc * 512 : (dc + 1) * 512])
                        sl[fi] = t_
                    for fi in fis:
                        for tt in range(TT):
                            nc.tensor.matmul(
                                pos[tt][:], hs_tiles[fi][:, tt * 128 : (tt + 1) * 128], sl[fi][:],
                                start=(idx[tt] == 0), stop=(idx[tt] == n_mm[tt] - 1),
                            )
                            idx[tt] += 1
                for tt in range(TT):
                    for ct in recv_pairs[tt]:
                        rt = rcv_tiles[ct]
                        nc.tensor.matmul(
                            pos[tt][:], ohr_sb[ct][:, tt * 128 : (tt + 1) * 128], rt[:],
                            start=(idx[tt] == 0), stop=(idx[tt] == n_mm[tt] - 1),
                        )
                        idx[tt] += 1
                for tt in range(TT):
                    osb = sb.tile([128, 512], F32, tag="osb", bufs=3, name=f"o{dc}_{tt}")
                    nc.scalar.activation(osb[:], pos[tt][:], AF.Copy)
                    nc.sync.dma_start(
                        out_ext[tt * 128 : (tt + 1) * 128, dc * 512 : (dc + 1) * 512], osb[:]
                    )

    nc.compile()
    return nc


_GRAPH_CACHE = {}
_LAST_RUN = None


def kernel(hidden_states, router_weight, w_gate, w_up, w_down, sw_gate, sw_up, sw_down):
    x = np.asarray(hidden_states, dtype=np.float32).reshape(T, D)
    rw = np.asarray(router_weight, dtype=np.float32)
    topk_idx, topk_w = _route(x, rw)

    # per-expert token/weight lists (token-ascending)
    tok = [[] for _ in range(E)]
    wt = [[] for _ in range(E)]
    for t in range(T):
        for k in range(TOPK):
            e = int(topk_idx[t, k])
            tok[e].append(t)
            wt[e].append(float(topk_w[t, k]))
    cnt = np.array([len(v) for v in tok])

    # pair big experts with small ones to minimize padded capacity
    order = np.argsort(-cnt, kind="stable")
    pair = [(int(order[i]), int(order[E - 1 - i])) for i in range(NC)]

    # per-local-expert capacities: el0 holds the big half, el1 the small half
    cmax = [max(cnt[pair[c][el]] for c in range(NC)) for el in range(EPC)]
    Cs = [int(np.ceil(max(64, m) / 64) * 64) for m in cmax]
    CTs = [(c + 127) // 128 for c in Cs]
    CT = sum(CTs)
    cto = [0, CTs[0]]
    C = max(Cs)

    paircnt = np.zeros((NC, NC), dtype=int)
    for c in range(NC):
        for e in pair[c]:
            for t in tok[e]:
                paircnt[c, t // TS] += 1
    P = int(np.ceil(max(1, paircnt.max()) / 16) * 16)
    NSLOT = NC * P
    NCT = NSLOT // 128

    # --- per-core gathered activations, combine weights, one-hot matrices ---
    xT = np.ascontiguousarray(x.T)  # [D, T]
    xg = np.zeros((NC, EPC, D, C), dtype=ml_dtypes.bfloat16)
    wvt = np.zeros((NC, 128, CT), dtype=np.float32)
    ohs = np.zeros((NC, EPC, max(CTs), 128, NSLOT), dtype=ml_dtypes.bfloat16)
    ohr = np.zeros((NC, NCT, 128, TS), dtype=ml_dtypes.bfloat16)
    for c in range(NC):
        fill = np.zeros(NC, dtype=int)
        for el in range(EPC):
            e = pair[c][el]
            tl = tok[e]
            if tl:
                xg[c, el, :, : len(tl)] = xT[:, tl].astype(ml_dtypes.bfloat16)
            for s_c, (t, w) in enumerate(zip(tl, wt[e])):
                wvt[c, s_c % 128, cto[el] + s_c // 128] = w
                dst = t // TS
                slab = dst * P + fill[dst]
                fill[dst] += 1
                ohs[c, el, s_c // 128, s_c % 128, slab] = 1.0
    # receiver view: core d's recv block s = what core s queued for dst d
    fill2 = np.zeros((NC, NC), dtype=int)
    for s in range(NC):
        for el in range(EPC):
            e = pair[s][el]
            for t in tok[e]:
                d = t // TS
                slot = s * P + fill2[s, d]
                fill2[s, d] += 1
                ohr[d, slot // 128, slot % 128, t - d * TS] = 1.0

    # union nonzero tile sets -> identical graph on every core
    send_pairs = [[] for _ in range(NCT)]
    for el in range(EPC):
        for ct in range(CTs[el]):
            nz = np.zeros(NCT, dtype=bool)
            for c in range(NC):
                v = (ohs[c, el, ct] != 0).reshape(128, NCT, 128).any(axis=(0, 2))
                nz |= v
            for st in np.where(nz)[0]:
                send_pairs[int(st)].append((el, ct))
    for st in range(NCT):
        if not send_pairs[st]:
            send_pairs[st].append((0, 0))  # all-zero one-hot: just zeros the slab tile
    recv_pairs = [[] for _ in range(TT)]
    for ct in range(NCT):
        nz = np.zeros(TT, dtype=bool)
        for c in range(NC):
            v = (ohr[c, ct] != 0).reshape(128, TT, 128).any(axis=(0, 2))
            nz |= v
        for tt in np.where(nz)[0]:
            recv_pairs[int(tt)].append(ct)

    # --- weight retiles (bf16) ---
    wg_t = (
        np.asarray(w_gate, np.float32)
        .reshape(E, DT, 128, FT, 128)
        .transpose(0, 3, 2, 1, 4)
        .astype(ml_dtypes.bfloat16)
    )  # [E, FT, 128(d_in), DT, 128(f_in)]
    wu_t = (
        np.asarray(w_up, np.float32)
        .reshape(E, DT, 128, FT, 128)
        .transpose(0, 3, 2, 1, 4)
        .astype(ml_dtypes.bfloat16)
    )
    wd_t = np.asarray(w_down, np.float32).reshape(E, FT, 128, D).astype(ml_dtypes.bfloat16)
    swg_t = (
        np.asarray(sw_gate, np.float32)
        .reshape(DT, 128, FST, 128)
        .transpose(2, 1, 0, 3)
        .astype(ml_dtypes.bfloat16)
    )  # [FST, 128(d_in), DT, 128(fs_in)]
    swu_t = (
        np.asarray(sw_up, np.float32)
        .reshape(DT, 128, FST, 128)
        .transpose(2, 1, 0, 3)
        .astype(ml_dtypes.bfloat16)
    )
    swd_t = np.asarray(sw_down, np.float32).reshape(FST, 128, D).astype(ml_dtypes.bfloat16)

    key = (
        tuple(Cs), NSLOT,
        tuple(tuple(p) for p in send_pairs), tuple(tuple(p) for p in recv_pairs),
    )
    nc = _GRAPH_CACHE.get(key)
    if nc is None:
        nc = _build(Cs, NSLOT, send_pairs, recv_pairs)
        _GRAPH_CACHE[key] = nc

    in_maps = []
    for c in range(NC):
        es = list(pair[c])
        in_maps.append(
            {
                "xg": np.ascontiguousarray(xg[c]),
                "xs": np.ascontiguousarray(xT[:, c * TS : (c + 1) * TS]).astype(
                    ml_dtypes.bfloat16
                ),
                "wg": np.ascontiguousarray(wg_t[es]),
                "wu": np.ascontiguousarray(wu_t[es]),
                "wd": np.ascontiguousarray(wd_t[es]),
                "swg": swg_t,
                "swu": swu_t,
                "swd": swd_t,
                "wvt": np.ascontiguousarray(wvt[c]),
                "ohs": np.ascontiguousarray(ohs[c]),
                "ohr": np.ascontiguousarray(ohr[c]),
            }
        )

    global _LAST_RUN
    _LAST_RUN = (nc, in_maps)
    res = run_bass_kernel_spmd(nc, in_maps, core_ids=list(range(NC)))
    out = np.concatenate([res.results[c]["out"] for c in range(NC)], axis=0)
    return out.reshape(1, T, D).astype(np.float32)


# revision 13
# speedup vs baseline: 1.0074x; 1.0074x over previous
"""AriaText MoE layer on 8 Trainium2 NeuronCores.

Strategy (expert-parallel + token-sharded shared expert):
- Host: router (softmax/top-4/renorm), per-expert token gather (pre-transposed
  activations), weight retile + bf16 cast, one-hot dispatch/combine matrices.
  Experts are paired onto cores big-with-small to minimize the padded
  capacity C.
- Device, per core (2 experts; token shard of 256):
  * gate/up/down GEMMs for the core's experts over their routed tokens
    (padded to capacity C), bf16 compute with fp32 PSUM accumulation;
    combine weight applied via per-partition scale on the PSUM->SBUF copy.
  * per 512-column slice: one-hot matmul reorders weighted expert rows into
    an AllToAll send slab [dst core][slot]; 5 column-sliced bf16 AllToAlls
    pipeline behind the down-projection so collective DMA traffic never
    stalls the TensorEngine.
  * shared expert computed token-sharded (full FS intermediate, 256 tokens).
  * final PSUM chain per output tile: shared-expert down-proj + one-hot
    scatter-add of received expert rows -> [256, 2560] f32 shard.
- Host concatenates the 8 shards into the full [1, 2048, 2560] output.
"""

import numpy as np
import ml_dtypes

import concourse.mybir as mybir
import concourse.tile as tile
from concourse import bacc
from concourse.bass_utils import run_bass_kernel_spmd

E, TOPK, D, F, FS = 16, 4, 2560, 1664, 3328
T = 2048
NC = 8
TS = T // NC  # tokens per core
EPC = E // NC  # experts per core
FT = F // 128  # 13
DT = D // 128  # 20
FST = FS // 128  # 26
DC = D // 512  # 5 output column chunks
TT = TS // 128  # 2 token tiles per core
BF16 = mybir.dt.bfloat16
F32 = mybir.dt.float32
AF = mybir.ActivationFunctionType


def _route(x32, router_weight):
    """Replicate reference routing (f64 for a stable top-k ordering)."""
    lg = x32.astype(np.float64) @ router_weight.astype(np.float64).T
    lg -= lg.max(-1, keepdims=True)
    p = np.exp(lg)
    p /= p.sum(-1, keepdims=True)
    idx = np.argsort(-p, axis=-1, kind="stable")[:, :TOPK]
    w = np.take_along_axis(p, idx, axis=-1)
    w = w / w.sum(-1, keepdims=True)
    return idx, w.astype(np.float32)


def _build(Cs, NSLOT, send_pairs, recv_pairs):
    """Build the SPMD graph. Cs = per-local-expert token capacities,
    NSLOT = A2A slab rows. send_pairs[st] = [(el, ct), ...];
    recv_pairs[tt] = [ct, ...] (union over cores, identical graph)."""
    C = max(Cs)
    CTs = [(c + 127) // 128 for c in Cs]
    CT = sum(CTs)
    cto = [0, CTs[0]]  # wvt column offset per el
    NCT = NSLOT // 128
    nc = bacc.Bacc("TRN2", target_bir_lowering=False, debug=False, num_devices=NC)

    xg_in = nc.dram_tensor("xg", [EPC, D, C], BF16, kind="ExternalInput")
    xs_in = nc.dram_tensor("xs", [D, TS], BF16, kind="ExternalInput")
    wg_in = nc.dram_tensor("wg", [EPC, FT, 128, DT, 128], BF16, kind="ExternalInput")
    wu_in = nc.dram_tensor("wu", [EPC, FT, 128, DT, 128], BF16, kind="ExternalInput")
    wd_in = nc.dram_tensor("wd", [EPC, FT, 128, D], BF16, kind="ExternalInput")
    swg_in = nc.dram_tensor("swg", [FST, 128, DT, 128], BF16, kind="ExternalInput")
    swu_in = nc.dram_tensor("swu", [FST, 128, DT, 128], BF16, kind="ExternalInput")
    swd_in = nc.dram_tensor("swd", [FST, 128, D], BF16, kind="ExternalInput")
    wvt_in = nc.dram_tensor("wvt", [128, CT], F32, kind="ExternalInput")
    ohs_in = nc.dram_tensor("ohs", [EPC, max(CTs), 128, NSLOT], BF16, kind="ExternalInput")
    ohr_in = nc.dram_tensor("ohr", [NCT, 128, TS], BF16, kind="ExternalInput")
    out_ext = nc.dram_tensor("out", [TS, D], F32, kind="ExternalOutput")

    chunks_el = []
    for c in Cs:
        ch = [(0, min(512, c))]
        if c > 512:
            ch.append((512, c - 512))
        chunks_el.append(ch)
    cth_el = [
        [min(128, c - ct * 128) for ct in range(n)] for c, n in zip(Cs, CTs)
    ]  # per-el per-c-tile height

    n_ohp = sum(len(p) for p in send_pairs)

    with tile.TileContext(nc) as tc:
        with (
            tc.tile_pool(name="sb", bufs=1) as sb,
            tc.tile_pool(name="ps", bufs=1, space="PSUM") as ps,
            tc.tile_pool(name="dr", bufs=1, space="DRAM") as dr,
        ):
            send_slabs = [
                dr.tile([NSLOT, 512], BF16, tag="slab", bufs=DC, name=f"sslab{dc}")
                for dc in range(DC)
            ]
            recv_slabs = [
                dr.tile([NSLOT, 512], BF16, tag="rslab", bufs=DC, name=f"rslab{dc}")
                for dc in range(DC)
            ]

            wvt_sb = sb.tile([128, CT], F32, tag="wvt", bufs=1, name="wvt")
            nc.sync.dma_start(wvt_sb[:], wvt_in[:])

            # tiny collective up-front: absorbs the first-collective
            # cross-core rendezvous off the critical path
            warm_s = dr.tile([NC, 512], BF16, tag="warm", bufs=2, name="warms")
            warm_r = dr.tile([NC, 512], BF16, tag="warm", bufs=2, name="warmr")
            warm_sb = sb.tile([NC, 512], BF16, tag="ssb", bufs=3, name="warmsb")
            nc.vector.memset(warm_sb[:], 0.0)
            nc.sync.dma_start(warm_s[:], warm_sb[:])
            nc.gpsimd.collective_compute(
                "AllToAll",
                mybir.AluOpType.bypass,
                replica_groups=[list(range(NC))],
                ins=[warm_s.opt()],
                outs=[warm_r.opt()],
            )

            # ---- phase 1a: gate/up for both experts ----
            h_tiles = {}  # (el, fi) -> tile [128, C_el]
            xg_sb = {}
            for el in range(EPC):
                Ce = Cs[el]
                chunks = chunks_el[el]
                # first weight slab lands before the xg block: shortens the head
                wgu_pre = {}
                wgt0 = sb.tile([128, DT, 128], BF16, tag="wgu", bufs=8, name=f"wg{el}_0")
                nc.sync.dma_start(wgt0[:], wg_in[el, 0])
                wut0 = sb.tile([128, DT, 128], BF16, tag="wgu", bufs=8, name=f"wu{el}_0")
                nc.sync.dma_start(wut0[:], wu_in[el, 0])
                wgu_pre[0] = (wgt0, wut0)
                for dt in range(DT):
                    t_ = sb.tile([128, Ce], BF16, tag="xgt", bufs=21, name=f"xg{el}_{dt}")
                    nc.sync.dma_start(t_[:], xg_in[el, dt * 128 : (dt + 1) * 128, :Ce])
                    xg_sb[(el, dt)] = t_
                for fi in range(FT):
                    if fi in wgu_pre:
                        wgt, wut = wgu_pre[fi]
                    else:
                        wgt = sb.tile([128, DT, 128], BF16, tag="wgu", bufs=8, name=f"wg{el}_{fi}")
                        nc.sync.dma_start(wgt[:], wg_in[el, fi])
                        wut = sb.tile([128, DT, 128], BF16, tag="wgu", bufs=8, name=f"wu{el}_{fi}")
                        nc.sync.dma_start(wut[:], wu_in[el, fi])
                    h_t = sb.tile([128, Ce], BF16, tag="h", bufs=2 * FT + 2, name=f"h{el}_{fi}")
                    pgs = [
                        ps.tile([128, 512], F32, tag="pgu", bufs=6, name=f"pg{el}_{fi}_{i}")
                        for i in range(len(chunks))
                    ]
                    pus = [
                        ps.tile([128, 512], F32, tag="pgu", bufs=6, name=f"pu{el}_{fi}_{i}")
                        for i in range(len(chunks))
                    ]
                    # interleave: same stationary drives all chunks back-to-back
                    for dt in range(DT):
                        st_, sp_ = (dt == 0), (dt == DT - 1)
                        for i, (off, cw) in enumerate(chunks):
                            nc.tensor.matmul(
                                pgs[i][:, :cw], wgt[:, dt, :],
                                xg_sb[(el, dt)][:, off : off + cw],
                                start=st_, stop=sp_,
                            )
                        for i, (off, cw) in enumerate(chunks):
                            nc.tensor.matmul(
                                pus[i][:, :cw], wut[:, dt, :],
                                xg_sb[(el, dt)][:, off : off + cw],
                                start=st_, stop=sp_,
                            )
                    for i, (off, cw) in enumerate(chunks):
                        sg = sb.tile([128, 512], BF16, tag="sg", bufs=3, name=f"sg{el}_{fi}_{i}")
                        nc.scalar.activation(sg[:, :cw], pgs[i][:, :cw], AF.Silu)
                        nc.vector.tensor_mul(h_t[:, off : off + cw], sg[:, :cw], pus[i][:, :cw])
                    h_tiles[(el, fi)] = h_t

            # ---- phase 1b: per column-slice: down-proj both experts,
            #      one-hot reorder into the send slab, column-sliced AllToAll ----
            ohp_tiles = {}
            for st in range(NCT):
                for el, ct in send_pairs[st]:
                    oh_t = sb.tile(
                        [128, 128], BF16, tag="ohp", bufs=n_ohp + 1,
                        name=f"ohp{st}_{el}_{ct}",
                    )
                    nc.sync.dma_start(oh_t[:], ohs_in[el, ct, :, st * 128 : (st + 1) * 128])
                    ohp_tiles[(st, el, ct)] = oh_t

            for dc in range(DC):
                y_sb = {}
                for el in range(EPC):
                    cth = cth_el[el]
                    wd_sl = []
                    for fi in range(FT):
                        t_ = sb.tile([128, 512], BF16, tag="wd", bufs=38, name=f"wd{el}_{fi}_{dc}")
                        nc.sync.dma_start(t_[:], wd_in[el, fi, :, dc * 512 : (dc + 1) * 512])
                        wd_sl.append(t_)
                    # interleave pairs of ct-chains so weight loads overlap matmuls
                    cts = list(range(CTs[el]))
                    for j in range(0, CTs[el], 2):
                        grp = cts[j : j + 2]
                        pys = {
                            ct: ps.tile([128, 512], F32, tag="py", bufs=2, name=f"py{el}_{dc}_{ct}")
                            for ct in grp
                        }
                        for fi in range(FT):
                            for ct in grp:
                                nc.tensor.matmul(
                                    pys[ct][: cth[ct], :],
                                    h_tiles[(el, fi)][:, ct * 128 : ct * 128 + cth[ct]],
                                    wd_sl[fi][:],
                                    start=(fi == 0), stop=(fi == FT - 1),
                                )
                        for ct in grp:
                            ysb = sb.tile([128, 512], BF16, tag="y", bufs=11, name=f"y{el}_{dc}_{ct}")
                            nc.scalar.activation(
                                ysb[: cth[ct], :], pys[ct][: cth[ct], :], AF.Copy,
                                scale=wvt_sb[: cth[ct], cto[el] + ct : cto[el] + ct + 1],
                            )
                            y_sb[(el, ct)] = ysb
                for st in range(NCT):
                    pairs = send_pairs[st]
                    ps_ = ps.tile([128, 512], F32, tag="py", bufs=2, name=f"pss{st}_{dc}")
                    for i, (el, ct) in enumerate(pairs):
                        h_ = cth_el[el][ct]
                        nc.tensor.matmul(
                            ps_[:], ohp_tiles[(st, el, ct)][:h_, :], y_sb[(el, ct)][:h_, :],
                            start=(i == 0), stop=(i == len(pairs) - 1),
                        )
                    ssb = sb.tile([128, 512], BF16, tag="ssb", bufs=3, name=f"ss{st}_{dc}")
                    nc.scalar.activation(ssb[:], ps_[:], AF.Copy)
                    nc.sync.dma_start(send_slabs[dc][st * 128 : (st + 1) * 128, :], ssb[:])
                nc.gpsimd.collective_compute(
                    "AllToAll",
                    mybir.AluOpType.bypass,
                    replica_groups=[list(range(NC))],
                    ins=[send_slabs[dc].opt()],
                    outs=[recv_slabs[dc].opt()],
                )

            # ---- phase 2: shared expert gate/up (token-sharded) ----
            xs_sb = sb.tile([128, DT, TS], BF16, tag="xs", bufs=1, name="xs")
            nc.sync.dma_start(xs_sb[:], xs_in.rearrange("(n p) t -> p n t", p=128))
            hs_tiles = []
            for fi in range(FST):
                sgt = sb.tile([128, DT, 128], BF16, tag="wgu", bufs=8, name=f"swg{fi}")
                nc.sync.dma_start(sgt[:], swg_in[fi])
                sut = sb.tile([128, DT, 128], BF16, tag="wgu", bufs=8, name=f"swu{fi}")
                nc.sync.dma_start(sut[:], swu_in[fi])
                pg = ps.tile([128, 512], F32, tag="pgu", bufs=6, name=f"psg{fi}")
                pu = ps.tile([128, 512], F32, tag="pgu", bufs=6, name=f"psu{fi}")
                for dt in range(DT):
                    st_, sp_ = (dt == 0), (dt == DT - 1)
                    nc.tensor.matmul(
                        pg[:, :TS], sgt[:, dt, :], xs_sb[:, dt, :], start=st_, stop=sp_
                    )
                    nc.tensor.matmul(
                        pu[:, :TS], sut[:, dt, :], xs_sb[:, dt, :], start=st_, stop=sp_
                    )
                sg = sb.tile([128, 512], BF16, tag="sg", bufs=3, name=f"ssg{fi}")
                nc.scalar.activation(sg[:, :TS], pg[:, :TS], AF.Silu)
                hs_t = sb.tile([128, TS], BF16, tag="hs", bufs=FST + 1, name=f"hs{fi}")
                nc.vector.tensor_mul(hs_t[:], sg[:, :TS], pu[:, :TS])
                hs_tiles.append(hs_t)

            # ---- phase 3: shared down-proj + scatter of received expert rows ----
            ohr_sb = []
            for ct in range(NCT):
                t_ = sb.tile([128, TS], BF16, tag="ohr", bufs=NCT + 1, name=f"ohr{ct}")
                nc.sync.dma_start(t_[:], ohr_in[ct])
                ohr_sb.append(t_)
            for dc in range(DC):
                rcv_tiles = {}
                for tt in range(TT):
                    for ct in recv_pairs[tt]:
                        if ct not in rcv_tiles:
                            rt = sb.tile([128, 512], BF16, tag="rcv", bufs=9, name=f"rcv{ct}_{dc}")
                            nc.sync.dma_start(
                                rt[:], recv_slabs[dc][ct * 128 : (ct + 1) * 128, :]
                            )
                            rcv_tiles[ct] = rt
                pos = {
                    tt: ps.tile([128, 512], F32, tag="py", bufs=2, name=f"po{dc}_{tt}")
                    for tt in range(TT)
                }
                n_mm = {tt: FST + len(recv_pairs[tt]) for tt in range(TT)}
                idx = {tt: 0 for tt in range(TT)}
                # swd slices in two halves to bound SBUF (shared "wd" tag)
                for half in range(2):
                    fis = range(13 * half, min(13 * (half + 1), FST))
                    sl = {}
                    for fi in fis:
                        t_ = sb.tile([128, 512], BF16, tag="wd", bufs=38, name=f"swd{fi}_{dc}")
                        nc.sync.dma_start(t_[:], swd_in[fi, :, dc * 512 : (dc + 1) * 512])
                        sl[fi] = t_
                    for fi in fis:
                        for tt in range(TT):
                            nc.tensor.matmul(
                                pos[tt][:], hs_tiles[fi][:, tt * 128 : (tt + 1) * 128], sl[fi][:],
                                start=(idx[tt] == 0), stop=(idx[tt] == n_mm[tt] - 1),
                            )
                            idx[tt] += 1
                for tt in range(TT):
                    for ct in recv_pairs[tt]:
                        rt = rcv_tiles[ct]
                        nc.tensor.matmul(
                            pos[tt][:], ohr_sb[ct][:, tt * 128 : (tt + 1) * 128], rt[:],
                            start=(idx[tt] == 0), stop=(idx[tt] == n_mm[tt] - 1),
                        )
                        idx[tt] += 1
                for tt in range(TT):
                    osb = sb.tile([128, 512], F32, tag="osb", bufs=4, name=f"o{dc}_{tt}")
                    nc.scalar.activation(osb[:], pos[tt][:], AF.Copy)
                    nc.sync.dma_start(
                        out_ext[tt * 128 : (tt + 1) * 128, dc * 512 : (dc + 1) * 512], osb[:]
                    )

    nc.compile()
    return nc


_GRAPH_CACHE = {}
_LAST_RUN = None


def kernel(hidden_states, router_weight, w_gate, w_up, w_down, sw_gate, sw_up, sw_down):
    x = np.asarray(hidden_states, dtype=np.float32).reshape(T, D)
    rw = np.asarray(router_weight, dtype=np.float32)
    topk_idx, topk_w = _route(x, rw)

    # per-expert token/weight lists (token-ascending)
    tok = [[] for _ in range(E)]
    wt = [[] for _ in range(E)]
    for t in range(T):
        for k in range(TOPK):
            e = int(topk_idx[t, k])
            tok[e].append(t)
            wt[e].append(float(topk_w[t, k]))
    cnt = np.array([len(v) for v in tok])

    # pair big experts with small ones to minimize padded capacity
    order = np.argsort(-cnt, kind="stable")
    pair = [(int(order[i]), int(order[E - 1 - i])) for i in range(NC)]

    # per-local-expert capacities: el0 holds the big half, el1 the small half
    cmax = [max(cnt[pair[c][el]] for c in range(NC)) for el in range(EPC)]
    Cs = [int(np.ceil(max(64, m) / 64) * 64) for m in cmax]
    CTs = [(c + 127) // 128 for c in Cs]
    CT = sum(CTs)
    cto = [0, CTs[0]]
    C = max(Cs)

    paircnt = np.zeros((NC, NC), dtype=int)
    for c in range(NC):
        for e in pair[c]:
            for t in tok[e]:
                paircnt[c, t // TS] += 1
    P = int(np.ceil(max(1, paircnt.max()) / 16) * 16)
    NSLOT = NC * P
    NCT = NSLOT // 128

    # --- per-core gathered activations, combine weights, one-hot matrices ---
    xT = np.ascontiguousarray(x.T)  # [D, T]
    xg = np.zeros((NC, EPC, D, C), dtype=ml_dtypes.bfloat16)
    wvt = np.zeros((NC, 128, CT), dtype=np.float32)
    ohs = np.zeros((NC, EPC, max(CTs), 128, NSLOT), dtype=ml_dtypes.bfloat16)
    ohr = np.zeros((NC, NCT, 128, TS), dtype=ml_dtypes.bfloat16)
    for c in range(NC):
        fill = np.zeros(NC, dtype=int)
        for el in range(EPC):
            e = pair[c][el]
            tl = tok[e]
            if tl:
                xg[c, el, :, : len(tl)] = xT[:, tl].astype(ml_dtypes.bfloat16)
            for s_c, (t, w) in enumerate(zip(tl, wt[e])):
                wvt[c, s_c % 128, cto[el] + s_c // 128] = w
                dst = t // TS
                slab = dst * P + fill[dst]
                fill[dst] += 1
                ohs[c, el, s_c // 128, s_c % 128, slab] = 1.0
    # receiver view: core d's recv block s = what core s queued for dst d
    fill2 = np.zeros((NC, NC), dtype=int)
    for s in range(NC):
        for el in range(EPC):
            e = pair[s][el]
            for t in tok[e]:
                d = t // TS
                slot = s * P + fill2[s, d]
                fill2[s, d] += 1
                ohr[d, slot // 128, slot % 128, t - d * TS] = 1.0

    # union nonzero tile sets -> identical graph on every core
    send_pairs = [[] for _ in range(NCT)]
    for el in range(EPC):
        for ct in range(CTs[el]):
            nz = np.zeros(NCT, dtype=bool)
            for c in range(NC):
                v = (ohs[c, el, ct] != 0).reshape(128, NCT, 128).any(axis=(0, 2))
                nz |= v
            for st in np.where(nz)[0]:
                send_pairs[int(st)].append((el, ct))
    for st in range(NCT):
        if not send_pairs[st]:
            send_pairs[st].append((0, 0))  # all-zero one-hot: just zeros the slab tile
    recv_pairs = [[] for _ in range(TT)]
    for ct in range(NCT):
        nz = np.zeros(TT, dtype=bool)
        for c in range(NC):
            v = (ohr[c, ct] != 0).reshape(128, TT, 128).any(axis=(0, 2))
            nz |= v
        for tt in np.where(nz)[0]:
            recv_pairs[int(tt)].append(ct)

    # --- weight retiles (bf16) ---
    wg_t = (
        np.asarray(w_gate, np.float32)
        .reshape(E, DT, 128, FT, 128)
        .transpose(0, 3, 2, 1, 4)
        .astype(ml_dtypes.bfloat16)
    )  # [E, FT, 128(d_in), DT, 128(f_in)]
    wu_t = (
        np.asarray(w_up, np.float32)
        .reshape(E, DT, 128, FT, 128)
        .transpose(0, 3, 2, 1, 4)
        .astype(ml_dtypes.bfloat16)
    )
    wd_t = np.asarray(w_down, np.float32).reshape(E, FT, 128, D).astype(ml_dtypes.bfloat16)
    swg_t = (
        np.asarray(sw_gate, np.float32)
        .reshape(DT, 128, FST, 128)
        .transpose(2, 1, 0, 3)
        .astype(ml_dtypes.bfloat16)
    )  # [FST, 128(d_in), DT, 128(fs_in)]
    swu_t = (
        np.asarray(sw_up, np.float32)
        .reshape(DT, 128, FST, 128)
        .transpose(2, 1, 0, 3)
        .astype(ml_dtypes.bfloat16)
    )
    swd_t = np.asarray(sw_down, np.float32).reshape(FST, 128, D).astype(ml_dtypes.bfloat16)

    key = (
        tuple(Cs), NSLOT,
        tuple(tuple(p) for p in send_pairs), tuple(tuple(p) for p in recv_pairs),
    )
    nc = _GRAPH_CACHE.get(key)
    if nc is None:
        nc = _build(Cs, NSLOT, send_pairs, recv_pairs)
        _GRAPH_CACHE[key] = nc

    in_maps = []
    for c in range(NC):
        es = list(pair[c])
        in_maps.append(
            {
                "xg": np.ascontiguousarray(xg[c]),
                "xs": np.ascontiguousarray(xT[:, c * TS : (c + 1) * TS]).astype(
                    ml_dtypes.bfloat16
                ),
                "wg": np.ascontiguousarray(wg_t[es]),
                "wu": np.ascontiguousarray(wu_t[es]),
                "wd": np.ascontiguousarray(wd_t[es]),
                "swg": swg_t,
                "swu": swu_t,
                "swd": swd_t,
                "wvt": np.ascontiguousarray(wvt[c]),
                "ohs": np.ascontiguousarray(ohs[c]),
                "ohr": np.ascontiguousarray(ohr[c]),
            }
        )

    global _LAST_RUN
    _LAST_RUN = (nc, in_maps)
    res = run_bass_kernel_spmd(nc, in_maps, core_ids=list(range(NC)))
    out = np.concatenate([res.results[c]["out"] for c in range(NC)], axis=0)
    return out.reshape(1, T, D).astype(np.float32)


# revision 14
# speedup vs baseline: 1.0519x; 1.0441x over previous
"""AriaText MoE layer on 8 Trainium2 NeuronCores.

Strategy (expert-parallel + token-sharded shared expert):
- Host: router (softmax/top-4/renorm), per-expert token gather (pre-transposed
  activations), weight retile + bf16 cast, one-hot dispatch/combine matrices.
  Experts are paired onto cores big-with-small to minimize the padded
  capacity C.
- Device, per core (2 experts; token shard of 256):
  * gate/up/down GEMMs for the core's experts over their routed tokens
    (padded to capacity C), bf16 compute with fp32 PSUM accumulation;
    combine weight applied via per-partition scale on the PSUM->SBUF copy.
  * per 512-column slice: one-hot matmul reorders weighted expert rows into
    an AllToAll send slab [dst core][slot]; 5 column-sliced bf16 AllToAlls
    pipeline behind the down-projection so collective DMA traffic never
    stalls the TensorEngine.
  * shared expert computed token-sharded (full FS intermediate, 256 tokens).
  * final PSUM chain per output tile: shared-expert down-proj + one-hot
    scatter-add of received expert rows -> [256, 2560] f32 shard.
- Host concatenates the 8 shards into the full [1, 2048, 2560] output.
"""

import numpy as np
import ml_dtypes

import concourse.mybir as mybir
import concourse.tile as tile
from concourse import bacc
from concourse.bass_utils import run_bass_kernel_spmd

E, TOPK, D, F, FS = 16, 4, 2560, 1664, 3328
T = 2048
NC = 8
TS = T // NC  # tokens per core
EPC = E // NC  # experts per core
FT = F // 128  # 13
DT = D // 128  # 20
FST = FS // 128  # 26
DC = D // 512  # 5 output column chunks
TT = TS // 128  # 2 token tiles per core
BF16 = mybir.dt.bfloat16
F32 = mybir.dt.float32
AF = mybir.ActivationFunctionType


def _route(x32, router_weight):
    """Replicate reference routing (f64 for a stable top-k ordering)."""
    lg = x32.astype(np.float64) @ router_weight.astype(np.float64).T
    lg -= lg.max(-1, keepdims=True)
    p = np.exp(lg)
    p /= p.sum(-1, keepdims=True)
    idx = np.argsort(-p, axis=-1, kind="stable")[:, :TOPK]
    w = np.take_along_axis(p, idx, axis=-1)
    w = w / w.sum(-1, keepdims=True)
    return idx, w.astype(np.float32)


def _build(Cs, NSLOT, send_pairs, recv_pairs):
    """Build the SPMD graph. Cs = per-local-expert token capacities,
    NSLOT = A2A slab rows. send_pairs[st] = [(el, ct), ...];
    recv_pairs[tt] = [ct, ...] (union over cores, identical graph)."""
    C = max(Cs)
    CTs = [(c + 127) // 128 for c in Cs]
    CT = sum(CTs)
    cto = [0, CTs[0]]  # wvt column offset per el
    NCT = NSLOT // 128
    nc = bacc.Bacc("TRN2", target_bir_lowering=False, debug=False, num_devices=NC)

    xg_in = nc.dram_tensor("xg", [EPC, D, C], BF16, kind="ExternalInput")
    xs_in = nc.dram_tensor("xs", [D, TS], BF16, kind="ExternalInput")
    wg_in = nc.dram_tensor("wg", [EPC, FT, 128, DT, 128], BF16, kind="ExternalInput")
    wu_in = nc.dram_tensor("wu", [EPC, FT, 128, DT, 128], BF16, kind="ExternalInput")
    wd_in = nc.dram_tensor("wd", [EPC, FT, 128, D], BF16, kind="ExternalInput")
    swg_in = nc.dram_tensor("swg", [FST, 128, DT, 128], BF16, kind="ExternalInput")
    swu_in = nc.dram_tensor("swu", [FST, 128, DT, 128], BF16, kind="ExternalInput")
    swd_in = nc.dram_tensor("swd", [FST, 128, D], BF16, kind="ExternalInput")
    wvt_in = nc.dram_tensor("wvt", [128, CT], F32, kind="ExternalInput")
    ohs_in = nc.dram_tensor("ohs", [EPC, max(CTs), 128, NSLOT], BF16, kind="ExternalInput")
    ohr_in = nc.dram_tensor("ohr", [NCT, 128, TS], BF16, kind="ExternalInput")
    out_ext = nc.dram_tensor("out", [TS, D], F32, kind="ExternalOutput")

    chunks_el = []
    for c in Cs:
        ch = [(0, min(512, c))]
        if c > 512:
            ch.append((512, c - 512))
        chunks_el.append(ch)
    cth_el = [
        [min(128, c - ct * 128) for ct in range(n)] for c, n in zip(Cs, CTs)
    ]  # per-el per-c-tile height

    n_ohp = sum(len(p) for p in send_pairs)

    with tile.TileContext(nc) as tc:
        with (
            tc.tile_pool(name="sb", bufs=1) as sb,
            tc.tile_pool(name="ps", bufs=1, space="PSUM") as ps,
            tc.tile_pool(name="dr", bufs=1, space="DRAM") as dr,
        ):
            send_slabs = [
                dr.tile([NSLOT, 512], BF16, tag="slab", bufs=DC, name=f"sslab{dc}")
                for dc in range(DC)
            ]
            recv_slabs = [
                dr.tile([NSLOT, 512], BF16, tag="rslab", bufs=DC, name=f"rslab{dc}")
                for dc in range(DC)
            ]

            wvt_sb = sb.tile([128, CT], F32, tag="wvt", bufs=1, name="wvt")
            nc.sync.dma_start(wvt_sb[:], wvt_in[:])

            # tiny collective up-front: absorbs the first-collective
            # cross-core rendezvous off the critical path
            warm_s = dr.tile([NC, 512], BF16, tag="warm", bufs=2, name="warms")
            warm_r = dr.tile([NC, 512], BF16, tag="warm", bufs=2, name="warmr")
            warm_sb = sb.tile([NC, 512], BF16, tag="ssb", bufs=3, name="warmsb")
            nc.vector.memset(warm_sb[:], 0.0)
            nc.sync.dma_start(warm_s[:], warm_sb[:])
            nc.gpsimd.collective_compute(
                "AllToAll",
                mybir.AluOpType.bypass,
                replica_groups=[list(range(NC))],
                ins=[warm_s.opt()],
                outs=[warm_r.opt()],
            )

            # ---- phase 1a: gate/up for both experts ----
            h_tiles = {}  # (el, fi) -> tile [128, C_el]
            xg_sb = {}
            for el in range(EPC):
                Ce = Cs[el]
                chunks = chunks_el[el]
                # first weight slab lands before the xg block: shortens the head
                wgu_pre = {}
                wgt0 = sb.tile([128, DT, 128], BF16, tag="wgu", bufs=7, name=f"wg{el}_0")
                nc.sync.dma_start(wgt0[:], wg_in[el, 0])
                wut0 = sb.tile([128, DT, 128], BF16, tag="wgu", bufs=7, name=f"wu{el}_0")
                nc.sync.dma_start(wut0[:], wu_in[el, 0])
                wgu_pre[0] = (wgt0, wut0)
                for dt in range(DT):
                    t_ = sb.tile([128, Ce], BF16, tag="xgt", bufs=21, name=f"xg{el}_{dt}")
                    nc.sync.dma_start(t_[:], xg_in[el, dt * 128 : (dt + 1) * 128, :Ce])
                    xg_sb[(el, dt)] = t_
                for fi in range(FT):
                    if fi in wgu_pre:
                        wgt, wut = wgu_pre[fi]
                    else:
                        wgt = sb.tile([128, DT, 128], BF16, tag="wgu", bufs=7, name=f"wg{el}_{fi}")
                        nc.sync.dma_start(wgt[:], wg_in[el, fi])
                        wut = sb.tile([128, DT, 128], BF16, tag="wgu", bufs=7, name=f"wu{el}_{fi}")
                        nc.sync.dma_start(wut[:], wu_in[el, fi])
                    h_t = sb.tile([128, Ce], BF16, tag="h", bufs=2 * FT + 2, name=f"h{el}_{fi}")
                    pgs = [
                        ps.tile([128, 512], F32, tag="pgu", bufs=6, name=f"pg{el}_{fi}_{i}")
                        for i in range(len(chunks))
                    ]
                    pus = [
                        ps.tile([128, 512], F32, tag="pgu", bufs=6, name=f"pu{el}_{fi}_{i}")
                        for i in range(len(chunks))
                    ]
                    # interleave: same stationary drives all chunks back-to-back
                    for dt in range(DT):
                        st_, sp_ = (dt == 0), (dt == DT - 1)
                        for i, (off, cw) in enumerate(chunks):
                            nc.tensor.matmul(
                                pgs[i][:, :cw], wgt[:, dt, :],
                                xg_sb[(el, dt)][:, off : off + cw],
                                start=st_, stop=sp_,
                            )
                        for i, (off, cw) in enumerate(chunks):
                            nc.tensor.matmul(
                                pus[i][:, :cw], wut[:, dt, :],
                                xg_sb[(el, dt)][:, off : off + cw],
                                start=st_, stop=sp_,
                            )
                    for i, (off, cw) in enumerate(chunks):
                        sg = sb.tile([128, 512], BF16, tag="sg", bufs=3, name=f"sg{el}_{fi}_{i}")
                        nc.scalar.activation(sg[:, :cw], pgs[i][:, :cw], AF.Silu)
                        nc.vector.tensor_mul(h_t[:, off : off + cw], sg[:, :cw], pus[i][:, :cw])
                    h_tiles[(el, fi)] = h_t

            # ---- phase 1b: per column-slice: down-proj both experts,
            #      one-hot reorder into the send slab, column-sliced AllToAll ----
            ohp_tiles = {}
            for st in range(NCT):
                for el, ct in send_pairs[st]:
                    oh_t = sb.tile(
                        [128, 128], BF16, tag="ohp", bufs=n_ohp + 1,
                        name=f"ohp{st}_{el}_{ct}",
                    )
                    nc.sync.dma_start(oh_t[:], ohs_in[el, ct, :, st * 128 : (st + 1) * 128])
                    ohp_tiles[(st, el, ct)] = oh_t

            for dc in range(DC):
                y_sb = {}
                for el in range(EPC):
                    cth = cth_el[el]
                    wd_sl = []
                    for fi in range(FT):
                        t_ = sb.tile([128, 512], BF16, tag="wd", bufs=41, name=f"wd{el}_{fi}_{dc}")
                        nc.sync.dma_start(t_[:], wd_in[el, fi, :, dc * 512 : (dc + 1) * 512])
                        wd_sl.append(t_)
                    # interleave pairs of ct-chains so weight loads overlap matmuls
                    cts = list(range(CTs[el]))
                    for j in range(0, CTs[el], 2):
                        grp = cts[j : j + 2]
                        pys = {
                            ct: ps.tile([128, 512], F32, tag="py", bufs=2, name=f"py{el}_{dc}_{ct}")
                            for ct in grp
                        }
                        for fi in range(FT):
                            for ct in grp:
                                nc.tensor.matmul(
                                    pys[ct][: cth[ct], :],
                                    h_tiles[(el, fi)][:, ct * 128 : ct * 128 + cth[ct]],
                                    wd_sl[fi][:],
                                    start=(fi == 0), stop=(fi == FT - 1),
                                )
                        for ct in grp:
                            ysb = sb.tile([128, 512], BF16, tag="y", bufs=11, name=f"y{el}_{dc}_{ct}")
                            nc.scalar.activation(
                                ysb[: cth[ct], :], pys[ct][: cth[ct], :], AF.Copy,
                                scale=wvt_sb[: cth[ct], cto[el] + ct : cto[el] + ct + 1],
                            )
                            y_sb[(el, ct)] = ysb
                for st in range(NCT):
                    pairs = send_pairs[st]
                    ps_ = ps.tile([128, 512], F32, tag="py", bufs=2, name=f"pss{st}_{dc}")
                    for i, (el, ct) in enumerate(pairs):
                        h_ = cth_el[el][ct]
                        nc.tensor.matmul(
                            ps_[:], ohp_tiles[(st, el, ct)][:h_, :], y_sb[(el, ct)][:h_, :],
                            start=(i == 0), stop=(i == len(pairs) - 1),
                        )
                    ssb = sb.tile([128, 512], BF16, tag="ssb", bufs=3, name=f"ss{st}_{dc}")
                    nc.scalar.activation(ssb[:], ps_[:], AF.Copy)
                    nc.sync.dma_start(send_slabs[dc][st * 128 : (st + 1) * 128, :], ssb[:])
                nc.gpsimd.collective_compute(
                    "AllToAll",
                    mybir.AluOpType.bypass,
                    replica_groups=[list(range(NC))],
                    ins=[send_slabs[dc].opt()],
                    outs=[recv_slabs[dc].opt()],
                )

            # ---- phase 2: shared expert gate/up (token-sharded) ----
            xs_sb = sb.tile([128, DT, TS], BF16, tag="xs", bufs=1, name="xs")
            nc.sync.dma_start(xs_sb[:], xs_in.rearrange("(n p) t -> p n t", p=128))
            hs_tiles = []
            for fi in range(FST):
                sgt = sb.tile([128, DT, 128], BF16, tag="wgu", bufs=7, name=f"swg{fi}")
                nc.sync.dma_start(sgt[:], swg_in[fi])
                sut = sb.tile([128, DT, 128], BF16, tag="wgu", bufs=7, name=f"swu{fi}")
                nc.sync.dma_start(sut[:], swu_in[fi])
                pg = ps.tile([128, 512], F32, tag="pgu", bufs=6, name=f"psg{fi}")
                pu = ps.tile([128, 512], F32, tag="pgu", bufs=6, name=f"psu{fi}")
                for dt in range(DT):
                    st_, sp_ = (dt == 0), (dt == DT - 1)
                    nc.tensor.matmul(
                        pg[:, :TS], sgt[:, dt, :], xs_sb[:, dt, :], start=st_, stop=sp_
                    )
                    nc.tensor.matmul(
                        pu[:, :TS], sut[:, dt, :], xs_sb[:, dt, :], start=st_, stop=sp_
                    )
                sg = sb.tile([128, 512], BF16, tag="sg", bufs=3, name=f"ssg{fi}")
                nc.scalar.activation(sg[:, :TS], pg[:, :TS], AF.Silu)
                hs_t = sb.tile([128, TS], BF16, tag="hs", bufs=FST + 1, name=f"hs{fi}")
                nc.vector.tensor_mul(hs_t[:], sg[:, :TS], pu[:, :TS])
                hs_tiles.append(hs_t)

            # ---- phase 3: shared down-proj + scatter of received expert rows ----
            ohr_sb = []
            for ct in range(NCT):
                t_ = sb.tile([128, TS], BF16, tag="ohr", bufs=NCT + 1, name=f"ohr{ct}")
                nc.sync.dma_start(t_[:], ohr_in[ct])
                ohr_sb.append(t_)
            for dc in range(DC):
                rcv_tiles = {}
                for tt in range(TT):
                    for ct in recv_pairs[tt]:
                        if ct not in rcv_tiles:
                            rt = sb.tile([128, 512], BF16, tag="rcv", bufs=12, name=f"rcv{ct}_{dc}")
                            nc.sync.dma_start(
                                rt[:], recv_slabs[dc][ct * 128 : (ct + 1) * 128, :]
                            )
                            rcv_tiles[ct] = rt
                pos = {
                    tt: ps.tile([128, 512], F32, tag="py", bufs=2, name=f"po{dc}_{tt}")
                    for tt in range(TT)
                }
                n_mm = {tt: FST + len(recv_pairs[tt]) for tt in range(TT)}
                idx = {tt: 0 for tt in range(TT)}
                # swd slices in two halves to bound SBUF (shared "wd" tag)
                for half in range(2):
                    fis = range(13 * half, min(13 * (half + 1), FST))
                    sl = {}
                    for fi in fis:
                        t_ = sb.tile([128, 512], BF16, tag="wd", bufs=41, name=f"swd{fi}_{dc}")
                        nc.sync.dma_start(t_[:], swd_in[fi, :, dc * 512 : (dc + 1) * 512])
                        sl[fi] = t_
                    for fi in fis:
                        for tt in range(TT):
                            nc.tensor.matmul(
                                pos[tt][:], hs_tiles[fi][:, tt * 128 : (tt + 1) * 128], sl[fi][:],
                                start=(idx[tt] == 0), stop=(idx[tt] == n_mm[tt] - 1),
                            )
                            idx[tt] += 1
                for tt in range(TT):
                    for ct in recv_pairs[tt]:
                        rt = rcv_tiles[ct]
                        nc.tensor.matmul(
                            pos[tt][:], ohr_sb[ct][:, tt * 128 : (tt + 1) * 128], rt[:],
                            start=(idx[tt] == 0), stop=(idx[tt] == n_mm[tt] - 1),
                        )
                        idx[tt] += 1
                for tt in range(TT):
                    osb = sb.tile([128, 512], F32, tag="osb", bufs=4, name=f"o{dc}_{tt}")
                    nc.scalar.activation(osb[:], pos[tt][:], AF.Copy)
                    nc.sync.dma_start(
                        out_ext[tt * 128 : (tt + 1) * 128, dc * 512 : (dc + 1) * 512], osb[:]
                    )

    nc.compile()
    return nc


_GRAPH_CACHE = {}
_LAST_RUN = None


def kernel(hidden_states, router_weight, w_gate, w_up, w_down, sw_gate, sw_up, sw_down):
    x = np.asarray(hidden_states, dtype=np.float32).reshape(T, D)
    rw = np.asarray(router_weight, dtype=np.float32)
    topk_idx, topk_w = _route(x, rw)

    # per-expert token/weight lists (token-ascending)
    tok = [[] for _ in range(E)]
    wt = [[] for _ in range(E)]
    for t in range(T):
        for k in range(TOPK):
            e = int(topk_idx[t, k])
            tok[e].append(t)
            wt[e].append(float(topk_w[t, k]))
    cnt = np.array([len(v) for v in tok])

    # pair big experts with small ones to minimize padded capacity
    order = np.argsort(-cnt, kind="stable")
    pair = [(int(order[i]), int(order[E - 1 - i])) for i in range(NC)]

    # per-local-expert capacities: el0 holds the big half, el1 the small half
    cmax = [max(cnt[pair[c][el]] for c in range(NC)) for el in range(EPC)]
    Cs = [int(np.ceil(max(64, m) / 64) * 64) for m in cmax]
    CTs = [(c + 127) // 128 for c in Cs]
    CT = sum(CTs)
    cto = [0, CTs[0]]
    C = max(Cs)

    paircnt = np.zeros((NC, NC), dtype=int)
    for c in range(NC):
        for e in pair[c]:
            for t in tok[e]:
                paircnt[c, t // TS] += 1
    P = int(np.ceil(max(1, paircnt.max()) / 16) * 16)
    NSLOT = NC * P
    NCT = NSLOT // 128

    # --- per-core gathered activations, combine weights, one-hot matrices ---
    xT = np.ascontiguousarray(x.T)  # [D, T]
    xg = np.zeros((NC, EPC, D, C), dtype=ml_dtypes.bfloat16)
    wvt = np.zeros((NC, 128, CT), dtype=np.float32)
    ohs = np.zeros((NC, EPC, max(CTs), 128, NSLOT), dtype=ml_dtypes.bfloat16)
    ohr = np.zeros((NC, NCT, 128, TS), dtype=ml_dtypes.bfloat16)
    for c in range(NC):
        fill = np.zeros(NC, dtype=int)
        for el in range(EPC):
            e = pair[c][el]
            tl = tok[e]
            if tl:
                xg[c, el, :, : len(tl)] = xT[:, tl].astype(ml_dtypes.bfloat16)
            for s_c, (t, w) in enumerate(zip(tl, wt[e])):
                wvt[c, s_c % 128, cto[el] + s_c // 128] = w
                dst = t // TS
                slab = dst * P + fill[dst]
                fill[dst] += 1
                ohs[c, el, s_c // 128, s_c % 128, slab] = 1.0
    # receiver view: core d's recv block s = what core s queued for dst d
    fill2 = np.zeros((NC, NC), dtype=int)
    for s in range(NC):
        for el in range(EPC):
            e = pair[s][el]
            for t in tok[e]:
                d = t // TS
                slot = s * P + fill2[s, d]
                fill2[s, d] += 1
                ohr[d, slot // 128, slot % 128, t - d * TS] = 1.0

    # union nonzero tile sets -> identical graph on every core
    send_pairs = [[] for _ in range(NCT)]
    for el in range(EPC):
        for ct in range(CTs[el]):
            nz = np.zeros(NCT, dtype=bool)
            for c in range(NC):
                v = (ohs[c, el, ct] != 0).reshape(128, NCT, 128).any(axis=(0, 2))
                nz |= v
            for st in np.where(nz)[0]:
                send_pairs[int(st)].append((el, ct))
    for st in range(NCT):
        if not send_pairs[st]:
            send_pairs[st].append((0, 0))  # all-zero one-hot: just zeros the slab tile
    recv_pairs = [[] for _ in range(TT)]
    for ct in range(NCT):
        nz = np.zeros(TT, dtype=bool)
        for c in range(NC):
            v = (ohr[c, ct] != 0).reshape(128, TT, 128).any(axis=(0, 2))
            nz |= v
        for tt in np.where(nz)[0]:
            recv_pairs[int(tt)].append(ct)

    # --- weight retiles (bf16) ---
    wg_t = (
        np.asarray(w_gate, np.float32)
        .reshape(E, DT, 128, FT, 128)
        .transpose(0, 3, 2, 1, 4)
        .astype(ml_dtypes.bfloat16)
    )  # [E, FT, 128(d_in), DT, 128(f_in)]
    wu_t = (
        np.asarray(w_up, np.float32)
        .reshape(E, DT, 128, FT, 128)
        .transpose(0, 3, 2, 1, 4)
        .astype(ml_dtypes.bfloat16)
    )
    wd_t = np.asarray(w_down, np.float32).reshape(E, FT, 128, D).astype(ml_dtypes.bfloat16)
    swg_t = (
        np.asarray(sw_gate, np.float32)
        .reshape(DT, 128, FST, 128)
        .transpose(2, 1, 0, 3)
        .astype(ml_dtypes.bfloat16)
    )  # [FST, 128(d_in), DT, 128(fs_in)]
    swu_t = (
        np.asarray(sw_up, np.float32)
        .reshape(DT, 128, FST, 128)
        .transpose(2, 1, 0, 3)
        .astype(ml_dtypes.bfloat16)
    )
    swd_t = np.asarray(sw_down, np.float32).reshape(FST, 128, D).astype(ml_dtypes.bfloat16)

    key = (
        tuple(Cs), NSLOT,
        tuple(tuple(p) for p in send_pairs), tuple(tuple(p) for p in recv_pairs),
    )
    nc = _GRAPH_CACHE.get(key)
    if nc is None:
        nc = _build(Cs, NSLOT, send_pairs, recv_pairs)
        _GRAPH_CACHE[key] = nc

    in_maps = []
    for c in range(NC):
        es = list(pair[c])
        in_maps.append(
            {
                "xg": np.ascontiguousarray(xg[c]),
                "xs": np.ascontiguousarray(xT[:, c * TS : (c + 1) * TS]).astype(
                    ml_dtypes.bfloat16
                ),
                "wg": np.ascontiguousarray(wg_t[es]),
                "wu": np.ascontiguousarray(wu_t[es]),
                "wd": np.ascontiguousarray(wd_t[es]),
                "swg": swg_t,
                "swu": swu_t,
                "swd": swd_t,
                "wvt": np.ascontiguousarray(wvt[c]),
                "ohs": np.ascontiguousarray(ohs[c]),
                "ohr": np.ascontiguousarray(ohr[c]),
            }
        )

    global _LAST_RUN
    _LAST_RUN = (nc, in_maps)
    res = run_bass_kernel_spmd(nc, in_maps, core_ids=list(range(NC)))
    out = np.concatenate([res.results[c]["out"] for c in range(NC)], axis=0)
    return out.reshape(1, T, D).astype(np.float32)


# revision 17
# speedup vs baseline: 1.0632x; 1.0108x over previous
"""AriaText MoE layer on 8 Trainium2 NeuronCores.

Strategy (expert-parallel + token-sharded shared expert):
- Host: router (softmax/top-4/renorm), per-expert token gather (pre-transposed
  activations), weight retile + bf16 cast, one-hot dispatch/combine matrices.
  Experts are paired onto cores big-with-small to minimize the padded
  capacity C.
- Device, per core (2 experts; token shard of 256):
  * gate/up/down GEMMs for the core's experts over their routed tokens
    (padded to capacity C), bf16 compute with fp32 PSUM accumulation;
    combine weight applied via per-partition scale on the PSUM->SBUF copy.
  * per 512-column slice: one-hot matmul reorders weighted expert rows into
    an AllToAll send slab [dst core][slot]; 5 column-sliced bf16 AllToAlls
    pipeline behind the down-projection so collective DMA traffic never
    stalls the TensorEngine.
  * shared expert computed token-sharded (full FS intermediate, 256 tokens).
  * final PSUM chain per output tile: shared-expert down-proj + one-hot
    scatter-add of received expert rows -> [256, 2560] f32 shard.
- Host concatenates the 8 shards into the full [1, 2048, 2560] output.
"""

import numpy as np
import ml_dtypes

import concourse.mybir as mybir
import concourse.tile as tile
from concourse import bacc
from concourse.bass_utils import run_bass_kernel_spmd

E, TOPK, D, F, FS = 16, 4, 2560, 1664, 3328
T = 2048
NC = 8
TS = T // NC  # tokens per core
EPC = E // NC  # experts per core
FT = F // 128  # 13
DT = D // 128  # 20
FST = FS // 128  # 26
DC = D // 512  # 5 output column chunks
TT = TS // 128  # 2 token tiles per core
BF16 = mybir.dt.bfloat16
F32 = mybir.dt.float32
AF = mybir.ActivationFunctionType


def _route(x32, router_weight):
    """Replicate reference routing (f64 for a stable top-k ordering)."""
    lg = x32.astype(np.float64) @ router_weight.astype(np.float64).T
    lg -= lg.max(-1, keepdims=True)
    p = np.exp(lg)
    p /= p.sum(-1, keepdims=True)
    idx = np.argsort(-p, axis=-1, kind="stable")[:, :TOPK]
    w = np.take_along_axis(p, idx, axis=-1)
    w = w / w.sum(-1, keepdims=True)
    return idx, w.astype(np.float32)


def _build(Cs, NSLOTs, send_pairs, recv_pairs):
    """Build the SPMD graph. Cs/NSLOTs = per-local-expert token capacities
    and A2A slab rows. send_pairs[el][st] = [ct, ...];
    recv_pairs[el][tt] = [ct, ...] (union over cores, identical graph)."""
    C = max(Cs)
    CTs = [(c + 127) // 128 for c in Cs]
    CT = sum(CTs)
    cto = [0, CTs[0]]  # wvt column offset per el
    NCTs = [n // 128 for n in NSLOTs]
    nc = bacc.Bacc("TRN2", target_bir_lowering=False, debug=False, num_devices=NC)

    xg_in = nc.dram_tensor("xg", [EPC, D, C], BF16, kind="ExternalInput")
    xs_in = nc.dram_tensor("xs", [D, TS], BF16, kind="ExternalInput")
    wg_in = nc.dram_tensor("wg", [EPC, FT, 128, DT, 128], BF16, kind="ExternalInput")
    wu_in = nc.dram_tensor("wu", [EPC, FT, 128, DT, 128], BF16, kind="ExternalInput")
    wd_in = nc.dram_tensor("wd", [EPC, FT, 128, D], BF16, kind="ExternalInput")
    swg_in = nc.dram_tensor("swg", [FST, 128, DT, 128], BF16, kind="ExternalInput")
    swu_in = nc.dram_tensor("swu", [FST, 128, DT, 128], BF16, kind="ExternalInput")
    swd_in = nc.dram_tensor("swd", [FST, 128, D], BF16, kind="ExternalInput")
    wvt_in = nc.dram_tensor("wvt", [128, CT], F32, kind="ExternalInput")
    ohs_ins = [
        nc.dram_tensor(f"ohs{el}", [CTs[el], 128, NSLOTs[el]], BF16, kind="ExternalInput")
        for el in range(EPC)
    ]
    ohr_ins = [
        nc.dram_tensor(f"ohr{el}", [NCTs[el], 128, TS], BF16, kind="ExternalInput")
        for el in range(EPC)
    ]
    out_ext = nc.dram_tensor("out", [TS, D], F32, kind="ExternalOutput")

    chunks_el = []
    for c in Cs:
        ch = [(0, min(512, c))]
        if c > 512:
            ch.append((512, c - 512))
        chunks_el.append(ch)
    cth_el = [
        [min(128, c - ct * 128) for ct in range(n)] for c, n in zip(Cs, CTs)
    ]  # per-el per-c-tile height

    n_ohp = sum(len(p) for el in range(EPC) for p in send_pairs[el])

    with tile.TileContext(nc) as tc:
        with (
            tc.tile_pool(name="sb", bufs=1) as sb,
            tc.tile_pool(name="ps", bufs=1, space="PSUM") as ps,
            tc.tile_pool(name="dr", bufs=1, space="DRAM") as dr,
        ):
            send_slabs = {
                (el, dc): dr.tile(
                    [NSLOTs[el], 512], BF16, tag="slab", bufs=EPC * DC,
                    name=f"sslab{el}_{dc}",
                )
                for el in range(EPC)
                for dc in range(DC)
            }
            recv_slabs = {
                (el, dc): dr.tile(
                    [NSLOTs[el], 512], BF16, tag="rslab", bufs=EPC * DC,
                    name=f"rslab{el}_{dc}",
                )
                for el in range(EPC)
                for dc in range(DC)
            }

            wvt_sb = sb.tile([128, CT], F32, tag="wvt", bufs=1, name="wvt")
            nc.sync.dma_start(wvt_sb[:], wvt_in[:])

            # tiny collective up-front: absorbs the first-collective
            # cross-core rendezvous off the critical path
            warm_s = dr.tile([NC, 512], BF16, tag="warm", bufs=2, name="warms")
            warm_r = dr.tile([NC, 512], BF16, tag="warm", bufs=2, name="warmr")
            warm_sb = sb.tile([NC, 512], BF16, tag="ssb", bufs=3, name="warmsb")
            nc.vector.memset(warm_sb[:], 0.0)
            nc.sync.dma_start(warm_s[:], warm_sb[:])
            nc.gpsimd.collective_compute(
                "AllToAll",
                mybir.AluOpType.bypass,
                replica_groups=[list(range(NC))],
                ins=[warm_s.opt()],
                outs=[warm_r.opt()],
            )

            # ---- phase 1: per expert: gate/up, then per column-slice
            #      down-proj -> one-hot reorder -> AllToAll (10 small A2As
            #      spread across the whole expert phase) ----
            xg_sb = {}
            for el in range(EPC):
                Ce = Cs[el]
                CTe = CTs[el]
                cth = cth_el[el]
                chunks = chunks_el[el]
                h_tiles = {}
                # first weight slab lands before the xg block: shortens the head
                wgu_pre = {}
                wgt0 = sb.tile([128, DT, 128], BF16, tag="wgu", bufs=7, name=f"wg{el}_0")
                nc.sync.dma_start(wgt0[:], wg_in[el, 0])
                wut0 = sb.tile([128, DT, 128], BF16, tag="wgu", bufs=7, name=f"wu{el}_0")
                nc.sync.dma_start(wut0[:], wu_in[el, 0])
                wgu_pre[0] = (wgt0, wut0)
                for dt in range(DT):
                    t_ = sb.tile([128, Ce], BF16, tag="xgt", bufs=21, name=f"xg{el}_{dt}")
                    nc.sync.dma_start(t_[:], xg_in[el, dt * 128 : (dt + 1) * 128, :Ce])
                    xg_sb[(el, dt)] = t_
                for fi in range(FT):
                    if fi in wgu_pre:
                        wgt, wut = wgu_pre[fi]
                    else:
                        wgt = sb.tile([128, DT, 128], BF16, tag="wgu", bufs=7, name=f"wg{el}_{fi}")
                        nc.sync.dma_start(wgt[:], wg_in[el, fi])
                        wut = sb.tile([128, DT, 128], BF16, tag="wgu", bufs=7, name=f"wu{el}_{fi}")
                        nc.sync.dma_start(wut[:], wu_in[el, fi])
                    h_t = sb.tile([128, Ce], BF16, tag="h", bufs=FT + 2, name=f"h{el}_{fi}")
                    pgs = [
                        ps.tile([128, 512], F32, tag="pgu", bufs=6, name=f"pg{el}_{fi}_{i}")
                        for i in range(len(chunks))
                    ]
                    pus = [
                        ps.tile([128, 512], F32, tag="pgu", bufs=6, name=f"pu{el}_{fi}_{i}")
                        for i in range(len(chunks))
                    ]
                    # interleave: same stationary drives all chunks back-to-back
                    for dt in range(DT):
                        st_, sp_ = (dt == 0), (dt == DT - 1)
                        for i, (off, cw) in enumerate(chunks):
                            nc.tensor.matmul(
                                pgs[i][:, :cw], wgt[:, dt, :],
                                xg_sb[(el, dt)][:, off : off + cw],
                                start=st_, stop=sp_,
                            )
                        for i, (off, cw) in enumerate(chunks):
                            nc.tensor.matmul(
                                pus[i][:, :cw], wut[:, dt, :],
                                xg_sb[(el, dt)][:, off : off + cw],
                                start=st_, stop=sp_,
                            )
                    for i, (off, cw) in enumerate(chunks):
                        sg = sb.tile([128, 512], BF16, tag="sg", bufs=3, name=f"sg{el}_{fi}_{i}")
                        nc.scalar.activation(sg[:, :cw], pgs[i][:, :cw], AF.Silu)
                        nc.vector.tensor_mul(h_t[:, off : off + cw], sg[:, :cw], pus[i][:, :cw])
                    h_tiles[fi] = h_t

                ohp_tiles = {}
                for st in range(NCTs[el]):
                    for ct in send_pairs[el][st]:
                        oh_t = sb.tile(
                            [128, 128], BF16, tag="ohp", bufs=n_ohp + 1,
                            name=f"ohp{el}_{st}_{ct}",
                        )
                        nc.sync.dma_start(
                            oh_t[:], ohs_ins[el][ct, :, st * 128 : (st + 1) * 128]
                        )
                        ohp_tiles[(st, ct)] = oh_t

                for dc in range(DC):
                    y_sb = {}
                    wd_sl = []
                    for fi in range(FT):
                        t_ = sb.tile([128, 512], BF16, tag="wd", bufs=41, name=f"wd{el}_{fi}_{dc}")
                        nc.sync.dma_start(t_[:], wd_in[el, fi, :, dc * 512 : (dc + 1) * 512])
                        wd_sl.append(t_)
                    # interleave pairs of ct-chains so weight loads overlap matmuls
                    cts = list(range(CTe))
                    for j in range(0, CTe, 2):
                        grp = cts[j : j + 2]
                        pys = {
                            ct: ps.tile([128, 512], F32, tag="py", bufs=2, name=f"py{el}_{dc}_{ct}")
                            for ct in grp
                        }
                        for fi in range(FT):
                            for ct in grp:
                                nc.tensor.matmul(
                                    pys[ct][: cth[ct], :],
                                    h_tiles[fi][:, ct * 128 : ct * 128 + cth[ct]],
                                    wd_sl[fi][:],
                                    start=(fi == 0), stop=(fi == FT - 1),
                                )
                        for ct in grp:
                            ysb = sb.tile([128, 512], BF16, tag="y", bufs=7, name=f"y{el}_{dc}_{ct}")
                            nc.scalar.activation(
                                ysb[: cth[ct], :], pys[ct][: cth[ct], :], AF.Copy,
                                scale=wvt_sb[: cth[ct], cto[el] + ct : cto[el] + ct + 1],
                            )
                            y_sb[ct] = ysb
                    for st in range(NCTs[el]):
                        pairs = send_pairs[el][st]
                        ps_ = ps.tile([128, 512], F32, tag="py", bufs=2, name=f"pss{el}_{st}_{dc}")
                        for i, ct in enumerate(pairs):
                            h_ = cth[ct]
                            nc.tensor.matmul(
                                ps_[:], ohp_tiles[(st, ct)][:h_, :], y_sb[ct][:h_, :],
                                start=(i == 0), stop=(i == len(pairs) - 1),
                            )
                        ssb = sb.tile([128, 512], BF16, tag="ssb", bufs=3, name=f"ss{el}_{st}_{dc}")
                        nc.scalar.activation(ssb[:], ps_[:], AF.Copy)
                        nc.sync.dma_start(
                            send_slabs[(el, dc)][st * 128 : (st + 1) * 128, :], ssb[:]
                        )
                    nc.gpsimd.collective_compute(
                        "AllToAll",
                        mybir.AluOpType.bypass,
                        replica_groups=[list(range(NC))],
                        ins=[send_slabs[(el, dc)].opt()],
                        outs=[recv_slabs[(el, dc)].opt()],
                    )

            # ---- phase 2: shared expert gate/up (token-sharded) ----
            xs_sb = sb.tile([128, DT, TS], BF16, tag="xs", bufs=1, name="xs")
            nc.sync.dma_start(xs_sb[:], xs_in.rearrange("(n p) t -> p n t", p=128))
            hs_tiles = []
            for fi in range(FST):
                sgt = sb.tile([128, DT, 128], BF16, tag="wgu", bufs=7, name=f"swg{fi}")
                nc.sync.dma_start(sgt[:], swg_in[fi])
                sut = sb.tile([128, DT, 128], BF16, tag="wgu", bufs=7, name=f"swu{fi}")
                nc.sync.dma_start(sut[:], swu_in[fi])
                pg = ps.tile([128, 512], F32, tag="pgu", bufs=6, name=f"psg{fi}")
                pu = ps.tile([128, 512], F32, tag="pgu", bufs=6, name=f"psu{fi}")
                for dt in range(DT):
                    st_, sp_ = (dt == 0), (dt == DT - 1)
                    nc.tensor.matmul(
                        pg[:, :TS], sgt[:, dt, :], xs_sb[:, dt, :], start=st_, stop=sp_
                    )
                    nc.tensor.matmul(
                        pu[:, :TS], sut[:, dt, :], xs_sb[:, dt, :], start=st_, stop=sp_
                    )
                sg = sb.tile([128, 512], BF16, tag="sg", bufs=3, name=f"ssg{fi}")
                nc.scalar.activation(sg[:, :TS], pg[:, :TS], AF.Silu)
                hs_t = sb.tile([128, TS], BF16, tag="hs", bufs=FST + 1, name=f"hs{fi}")
                nc.vector.tensor_mul(hs_t[:], sg[:, :TS], pu[:, :TS])
                hs_tiles.append(hs_t)

            # ---- phase 3: shared down-proj + scatter of received expert rows ----
            ohr_sb = {}
            for el in range(EPC):
                for ct in range(NCTs[el]):
                    t_ = sb.tile(
                        [128, TS], BF16, tag="ohr", bufs=sum(NCTs) + 1, name=f"ohr{el}_{ct}"
                    )
                    nc.sync.dma_start(t_[:], ohr_ins[el][ct])
                    ohr_sb[(el, ct)] = t_
            for dc in range(DC):
                rcv_tiles = {}
                for el in range(EPC):
                    for tt in range(TT):
                        for ct in recv_pairs[el][tt]:
                            if (el, ct) not in rcv_tiles:
                                rt = sb.tile(
                                    [128, 512], BF16, tag="rcv", bufs=14, name=f"rcv{el}_{ct}_{dc}"
                                )
                                nc.sync.dma_start(
                                    rt[:], recv_slabs[(el, dc)][ct * 128 : (ct + 1) * 128, :]
                                )
                                rcv_tiles[(el, ct)] = rt
                pos = {
                    tt: ps.tile([128, 512], F32, tag="py", bufs=2, name=f"po{dc}_{tt}")
                    for tt in range(TT)
                }
                n_mm = {
                    tt: FST + sum(len(recv_pairs[el][tt]) for el in range(EPC))
                    for tt in range(TT)
                }
                idx = {tt: 0 for tt in range(TT)}
                # swd slices in two halves to bound SBUF (shared "wd" tag)
                for half in range(2):
                    fis = range(13 * half, min(13 * (half + 1), FST))
                    sl = {}
                    for fi in fis:
                        t_ = sb.tile([128, 512], BF16, tag="wd", bufs=41, name=f"swd{fi}_{dc}")
                        nc.sync.dma_start(t_[:], swd_in[fi, :, dc * 512 : (dc + 1) * 512])
                        sl[fi] = t_
                    for fi in fis:
                        for tt in range(TT):
                            nc.tensor.matmul(
                                pos[tt][:], hs_tiles[fi][:, tt * 128 : (tt + 1) * 128], sl[fi][:],
                                start=(idx[tt] == 0), stop=(idx[tt] == n_mm[tt] - 1),
                            )
                            idx[tt] += 1
                for tt in range(TT):
                    for el in range(EPC):
                        for ct in recv_pairs[el][tt]:
                            rt = rcv_tiles[(el, ct)]
                            nc.tensor.matmul(
                                pos[tt][:], ohr_sb[(el, ct)][:, tt * 128 : (tt + 1) * 128], rt[:],
                                start=(idx[tt] == 0), stop=(idx[tt] == n_mm[tt] - 1),
                            )
                            idx[tt] += 1
                for tt in range(TT):
                    osb = sb.tile([128, 512], F32, tag="osb", bufs=4, name=f"o{dc}_{tt}")
                    nc.scalar.activation(osb[:], pos[tt][:], AF.Copy)
                    nc.sync.dma_start(
                        out_ext[tt * 128 : (tt + 1) * 128, dc * 512 : (dc + 1) * 512], osb[:]
                    )

    nc.compile()
    return nc


_GRAPH_CACHE = {}
_LAST_RUN = None


def kernel(hidden_states, router_weight, w_gate, w_up, w_down, sw_gate, sw_up, sw_down):
    x = np.asarray(hidden_states, dtype=np.float32).reshape(T, D)
    rw = np.asarray(router_weight, dtype=np.float32)
    topk_idx, topk_w = _route(x, rw)

    # per-expert token/weight lists (token-ascending)
    tok = [[] for _ in range(E)]
    wt = [[] for _ in range(E)]
    for t in range(T):
        for k in range(TOPK):
            e = int(topk_idx[t, k])
            tok[e].append(t)
            wt[e].append(float(topk_w[t, k]))
    cnt = np.array([len(v) for v in tok])

    # pair big experts with small ones to minimize padded capacity
    order = np.argsort(-cnt, kind="stable")
    pair = [(int(order[i]), int(order[E - 1 - i])) for i in range(NC)]

    # per-local-expert capacities: el0 holds the big half, el1 the small half
    cmax = [max(cnt[pair[c][el]] for c in range(NC)) for el in range(EPC)]
    Cs = [int(np.ceil(max(64, m) / 64) * 64) for m in cmax]
    CTs = [(c + 127) // 128 for c in Cs]
    CT = sum(CTs)
    cto = [0, CTs[0]]
    C = max(Cs)

    paircnt = np.zeros((EPC, NC, NC), dtype=int)
    for c in range(NC):
        for el in range(EPC):
            for t in tok[pair[c][el]]:
                paircnt[el, c, t // TS] += 1
    Ps = [int(np.ceil(max(1, paircnt[el].max()) / 16) * 16) for el in range(EPC)]
    NSLOTs = [NC * p for p in Ps]
    NCTs_s = [n // 128 for n in NSLOTs]

    # --- per-core gathered activations, combine weights, one-hot matrices ---
    xT = np.ascontiguousarray(x.T)  # [D, T]
    xg = np.zeros((NC, EPC, D, C), dtype=ml_dtypes.bfloat16)
    wvt = np.zeros((NC, 128, CT), dtype=np.float32)
    ohs = [
        np.zeros((NC, CTs[el], 128, NSLOTs[el]), dtype=ml_dtypes.bfloat16)
        for el in range(EPC)
    ]
    ohr = [
        np.zeros((NC, NCTs_s[el], 128, TS), dtype=ml_dtypes.bfloat16)
        for el in range(EPC)
    ]
    for c in range(NC):
        for el in range(EPC):
            fill = np.zeros(NC, dtype=int)
            e = pair[c][el]
            tl = tok[e]
            if tl:
                xg[c, el, :, : len(tl)] = xT[:, tl].astype(ml_dtypes.bfloat16)
            for s_c, (t, w) in enumerate(zip(tl, wt[e])):
                wvt[c, s_c % 128, cto[el] + s_c // 128] = w
                dst = t // TS
                slab = dst * Ps[el] + fill[dst]
                fill[dst] += 1
                ohs[el][c, s_c // 128, s_c % 128, slab] = 1.0
    # receiver view: core d's recv block s = what core s queued for dst d
    for s in range(NC):
        for el in range(EPC):
            fill2 = np.zeros(NC, dtype=int)
            e = pair[s][el]
            for t in tok[e]:
                d = t // TS
                slot = s * Ps[el] + fill2[d]
                fill2[d] += 1
                ohr[el][d, slot // 128, slot % 128, t - d * TS] = 1.0

    # union nonzero tile sets -> identical graph on every core
    send_pairs = [[[] for _ in range(NCTs_s[el])] for el in range(EPC)]
    for el in range(EPC):
        for ct in range(CTs[el]):
            nz = np.zeros(NCTs_s[el], dtype=bool)
            for c in range(NC):
                v = (ohs[el][c, ct] != 0).reshape(128, NCTs_s[el], 128).any(axis=(0, 2))
                nz |= v
            for st in np.where(nz)[0]:
                send_pairs[el][int(st)].append(ct)
        for st in range(NCTs_s[el]):
            if not send_pairs[el][st]:
                send_pairs[el][st].append(0)  # all-zero one-hot: zeros the slab tile
    recv_pairs = [[[] for _ in range(TT)] for el in range(EPC)]
    for el in range(EPC):
        for ct in range(NCTs_s[el]):
            nz = np.zeros(TT, dtype=bool)
            for c in range(NC):
                v = (ohr[el][c, ct] != 0).reshape(128, TT, 128).any(axis=(0, 2))
                nz |= v
            for tt in np.where(nz)[0]:
                recv_pairs[el][int(tt)].append(ct)

    # --- weight retiles (bf16) ---
    wg_t = (
        np.asarray(w_gate, np.float32)
        .reshape(E, DT, 128, FT, 128)
        .transpose(0, 3, 2, 1, 4)
        .astype(ml_dtypes.bfloat16)
    )  # [E, FT, 128(d_in), DT, 128(f_in)]
    wu_t = (
        np.asarray(w_up, np.float32)
        .reshape(E, DT, 128, FT, 128)
        .transpose(0, 3, 2, 1, 4)
        .astype(ml_dtypes.bfloat16)
    )
    wd_t = np.asarray(w_down, np.float32).reshape(E, FT, 128, D).astype(ml_dtypes.bfloat16)
    swg_t = (
        np.asarray(sw_gate, np.float32)
        .reshape(DT, 128, FST, 128)
        .transpose(2, 1, 0, 3)
        .astype(ml_dtypes.bfloat16)
    )  # [FST, 128(d_in), DT, 128(fs_in)]
    swu_t = (
        np.asarray(sw_up, np.float32)
        .reshape(DT, 128, FST, 128)
        .transpose(2, 1, 0, 3)
        .astype(ml_dtypes.bfloat16)
    )
    swd_t = np.asarray(sw_down, np.float32).reshape(FST, 128, D).astype(ml_dtypes.bfloat16)

    key = (
        tuple(Cs), tuple(NSLOTs),
        tuple(tuple(tuple(p) for p in sp) for sp in send_pairs),
        tuple(tuple(tuple(p) for p in rp) for rp in recv_pairs),
    )
    nc = _GRAPH_CACHE.get(key)
    if nc is None:
        nc = _build(Cs, NSLOTs, send_pairs, recv_pairs)
        _GRAPH_CACHE[key] = nc

    in_maps = []
    for c in range(NC):
        es = list(pair[c])
        in_maps.append(
            {
                "xg": np.ascontiguousarray(xg[c]),
                "xs": np.ascontiguousarray(xT[:, c * TS : (c + 1) * TS]).astype(
                    ml_dtypes.bfloat16
                ),
                "wg": np.ascontiguousarray(wg_t[es]),
                "wu": np.ascontiguousarray(wu_t[es]),
                "wd": np.ascontiguousarray(wd_t[es]),
                "swg": swg_t,
                "swu": swu_t,
                "swd": swd_t,
                "wvt": np.ascontiguousarray(wvt[c]),
                "ohs0": np.ascontiguousarray(ohs[0][c]),
                "ohs1": np.ascontiguousarray(ohs[1][c]),
                "ohr0": np.ascontiguousarray(ohr[0][c]),
                "ohr1": np.ascontiguousarray(ohr[1][c]),
            }
        )

    global _LAST_RUN
    _LAST_RUN = (nc, in_maps)
    res = run_bass_kernel_spmd(nc, in_maps, core_ids=list(range(NC)))
    out = np.concatenate([res.results[c]["out"] for c in range(NC)], axis=0)
    return out.reshape(1, T, D).astype(np.float32)
